# revision 81
# baseline (speedup 1.0000x reference)
"""GPT block (LN -> causal MHA -> LN -> MLP) on 8 TRN2 NeuronCores.

Sharding: each core owns one (batch, query-chunk-pair). B=4 batches x 2
chunk-pairs = 8 cores. Chunk pairs are zig-zag ({0,3} / {1,2}) over four
512-row chunks of T=2048 so attention work balances. Each core recomputes
K/V for the full sequence locally (no collectives), runs flash-style
attention for its 1024 query rows, then the MLP for the same rows.

All activations live feature-on-partition ("transposed"); the host
pre-transposes x and assembles the output, so no on-chip transposes are
needed. Per-core causality is handled with a block permutation of the
sequence: each core sees [own-chunk-A | own-chunk-B | other | other], so
the program is identical across cores; the diagonal-block masks are
static and full-block allow/deny is driven by per-core exp scale/bias
inputs (exp(0*s - 1e9) = 0 kills forbidden blocks).
"""

import numpy as np
import ml_dtypes

BF = ml_dtypes.bfloat16
F8 = ml_dtypes.float8_e4m3

E = 1024          # embedding
T = 2048          # sequence
B = 4             # batch
NH = 16           # heads
D = 64            # head dim
HID = 4096        # mlp hidden
KT = E // 128     # k-tiles over embedding (8)
CH = 512          # chunk rows
NEG = -1.0e9
EPS = 1e-5

_CACHE = {}


def _build_program():
    import concourse.bass as bass
    import concourse.tile as tile
    from concourse import bacc, mybir

    f32 = mybir.dt.float32
    bf16 = mybir.dt.bfloat16
    f8 = mybir.dt.float8e4
    AF = mybir.ActivationFunctionType
    ALU = mybir.AluOpType
    DR = mybir.MatmulPerfMode.DoubleRow

    nc = bacc.Bacc()

    xT_d = nc.declare_dram_parameter("xT", [E, T], f32, isOutput=False)
    xTb_d = nc.declare_dram_parameter("xTb", [E, T], bf16, isOutput=False)
    w_attn_d = nc.declare_dram_parameter("w_attn", [E, 3 * E], f8, isOutput=False)
    b_qk_d = nc.declare_dram_parameter("b_qk", [2 * E, 1], f32, isOutput=False)
    b_v_d = nc.declare_dram_parameter("b_v", [1, E], f32, isOutput=False)
    w_ap_d = nc.declare_dram_parameter("w_ap", [E, E], bf16, isOutput=False)
    b_ap_d = nc.declare_dram_parameter("b_ap", [E, 1], f32, isOutput=False)
    ln1_g_d = nc.declare_dram_parameter("ln1_g", [E, 1], f32, isOutput=False)
    ln1_b_d = nc.declare_dram_parameter("ln1_b", [E, 1], f32, isOutput=False)
    ln2_g_d = nc.declare_dram_parameter("ln2_g", [E, 1], f32, isOutput=False)
    ln2_b_d = nc.declare_dram_parameter("ln2_b", [E, 1], f32, isOutput=False)
    w_fc_d = nc.declare_dram_parameter("w_fc", [E, HID], bf16, isOutput=False)
    b_fc_d = nc.declare_dram_parameter("b_fc", [HID, 1], f32, isOutput=False)
    w_pr_d = nc.declare_dram_parameter("w_proj", [HID, E], bf16, isOutput=False)
    b_pr_d = nc.declare_dram_parameter("b_proj", [E, 1], f32, isOutput=False)
    dmask_d = nc.declare_dram_parameter("dmask", [4, 128, CH], bf16, isOutput=False)
    sA_s_d = nc.declare_dram_parameter("sA_scale", [128, 1], f32, isOutput=False)
    sA_b_d = nc.declare_dram_parameter("sA_bias", [128, 1], f32, isOutput=False)
    sB_s_d = nc.declare_dram_parameter("sB_scale", [128, 3], f32, isOutput=False)
    sB_b_d = nc.declare_dram_parameter("sB_bias", [128, 3], f32, isOutput=False)
    out_d = nc.declare_dram_parameter("outT", [E, 2 * CH], f32, isOutput=True)

    with tile.TileContext(nc) as tc:
        from contextlib import ExitStack

        stack = ExitStack()
        with stack:
            const = stack.enter_context(tc.tile_pool(name="const", bufs=1))

            ones_col_bf = const.tile([128, 1], bf16)
            nc.vector.memset(ones_col_bf[:], 1.0)
            ones_row_f = const.tile([1, 128], f32)
            nc.vector.memset(ones_row_f[:], 1.0)
            ones_row_bf = const.tile([1, 64], bf16)
            nc.vector.memset(ones_row_bf[:], 1.0)
            ones_ch_bf = const.tile([1, CH], bf16)
            nc.vector.memset(ones_ch_bf[:], 1.0)
            eps_t = const.tile([1, 1], f32)
            nc.vector.memset(eps_t[:], EPS)

            dmask = const.tile([128, 4, CH], bf16)
            nc.sync.dma_start(dmask[:], dmask_d.rearrange("v p n -> p v n"))
            sA_s = const.tile([128, 1], f32)
            nc.sync.dma_start(sA_s[:], sA_s_d[:])
            sA_b = const.tile([128, 1], f32)
            nc.sync.dma_start(sA_b[:], sA_b_d[:])
            sB_s = const.tile([128, 3], f32)
            nc.sync.dma_start(sB_s[:], sB_s_d[:])
            sB_b = const.tile([128, 3], f32)
            nc.sync.dma_start(sB_b[:], sB_b_d[:])

            ln1_g = const.tile([128, KT, 1], f32)
            nc.sync.dma_start(ln1_g[:], ln1_g_d.rearrange("(k p) o -> p k o", p=128))
            ln1_b = const.tile([128, KT, 1], f32)
            nc.sync.dma_start(ln1_b[:], ln1_b_d.rearrange("(k p) o -> p k o", p=128))
            ln2_g = const.tile([128, KT, 1], f32)
            nc.sync.dma_start(ln2_g[:], ln2_g_d.rearrange("(k p) o -> p k o", p=128))
            ln2_b = const.tile([128, KT, 1], f32)
            nc.sync.dma_start(ln2_b[:], ln2_b_d.rearrange("(k p) o -> p k o", p=128))
            b_qk = const.tile([128, 16, 1], f32)
            nc.sync.dma_start(b_qk[:], b_qk_d.rearrange("(k p) o -> p k o", p=128))
            b_v = const.tile([1, E], f32)
            nc.sync.dma_start(b_v[:], b_v_d[:])
            b_ap = const.tile([128, KT, 1], f32)
            nc.sync.dma_start(b_ap[:], b_ap_d.rearrange("(k p) o -> p k o", p=128))
            b_fc = const.tile([128, 32, 1], f32)
            nc.sync.dma_start(b_fc[:], b_fc_d.rearrange("(k p) o -> p k o", p=128))
            b_pr = const.tile([128, KT, 1], f32)
            nc.sync.dma_start(b_pr[:], b_pr_d.rearrange("(k p) o -> p k o", p=128))

            def layernorm(dst, src_fn, ncols, g_t, b_t, chunks=None):
                """dst[:, kt, c] = LN over feature dim of src columns.

                src_fn(kt, ch) -> ("dma_bf", dram bf16 AP) | ("sbuf_f32", AP)
                All elementwise work runs in bf16 (DVE 2x mode).
                """
                if chunks is None:
                    chunks = range(ncols // CH)
                with tc.tile_pool(name="lnp", bufs=2, space="PSUM") as lnps, \
                     tc.tile_pool(name="lns", bufs=3) as lnsb:
                    for ch in chunks:
                        kind0, bulk = src_fn(0, ch)
                        if kind0 == "dma_bulk":
                            # one strided DMA for the whole chunk
                            xbig = lnsb.tile([128, KT, CH], bf16, tag="xbig",
                                             bufs=2)
                            nc.sync.dma_start(xbig[:], bulk)
                        xbfs = []
                        mu_ps = lnps.tile([1, CH], f32, tag="stat")
                        ss_ps = lnps.tile([1, CH], f32, tag="stat")
                        for kt in range(KT):
                            if kind0 == "dma_bulk":
                                xbf = xbig[:, kt, :]
                            else:
                                kind, src = src_fn(kt, ch)
                                xbf = lnsb.tile([128, CH], bf16, tag="xbf",
                                                bufs=10)
                                nc.scalar.copy(xbf[:], src)
                            xbfs.append(xbf)
                            sq = lnsb.tile([128, CH], bf16, tag="sq")
                            nc.vector.tensor_mul(sq[:], xbf[:], xbf[:])
                            nc.tensor.matmul(mu_ps[:], ones_col_bf[:], xbf[:],
                                             start=(kt == 0), stop=(kt == KT - 1))
                            nc.tensor.matmul(ss_ps[:], ones_col_bf[:], sq[:],
                                             start=(kt == 0), stop=(kt == KT - 1))
                        # rows: musq = (S1/E)^2; var = S2/E - musq;
                        # a = 1/sqrt(var+eps); c = (-S1/E)*a
                        musq = lnsb.tile([1, CH], f32, tag="row", bufs=8)
                        nc.scalar.activation(musq[:], mu_ps[:], AF.Square,
                                             scale=1.0 / E)
                        var = lnsb.tile([1, CH], f32, tag="row", bufs=8)
                        nc.vector.scalar_tensor_tensor(
                            var[:], ss_ps[:], 1.0 / E, musq[:],
                            ALU.mult, ALU.subtract)
                        sd = lnsb.tile([1, CH], f32, tag="row", bufs=8)
                        nc.scalar.activation(sd[:], var[:], AF.Sqrt,
                                             bias=eps_t[:])
                        a_row = lnsb.tile([1, CH], f32, tag="row", bufs=8)
                        nc.vector.reciprocal_approx_fast(out=a_row[:], in_=sd[:])
                        c_row = lnsb.tile([1, CH], f32, tag="row", bufs=8)
                        nc.vector.scalar_tensor_tensor(
                            c_row[:], mu_ps[:], -1.0 / E, a_row[:],
                            ALU.mult, ALU.mult)
                        a_bc = lnps.tile([128, CH], f32, tag="bc")
                        nc.tensor.matmul(a_bc[:], ones_row_f[:], a_row[:],
                                         start=True, stop=True)
                        c_bc = lnps.tile([128, CH], f32, tag="bc")
                        nc.tensor.matmul(c_bc[:], ones_row_f[:], c_row[:],
                                         start=True, stop=True)
                        a_sb = lnsb.tile([128, CH], bf16, tag="asb")
                        nc.vector.tensor_copy(a_sb[:], a_bc[:])
                        c_sb = lnsb.tile([128, CH], bf16, tag="csb")
                        nc.vector.tensor_copy(c_sb[:], c_bc[:])
                        for kt in range(KT):
                            t1 = lnsb.tile([128, CH], bf16, tag="t1")
                            nc.vector.tensor_mul(t1[:], xbfs[kt][:], a_sb[:])
                            t2 = lnsb.tile([128, CH], bf16, tag="t2")
                            nc.vector.tensor_add(t2[:], t1[:], c_sb[:])
                            # gain/bias apply on ScalarE (idle during LN)
                            nc.scalar.activation(
                                dst[:, kt, ch * CH:(ch + 1) * CH], t2[:],
                                AF.Identity, bias=b_t[:, kt, 0:1],
                                scale=g_t[:, kt, 0:1])

            # ---------------- Phase 1+2: LN1 and QKV ----------------
            ln1_ctx = ExitStack()
            ln1 = ln1_ctx.enter_context(tc.tile_pool(name="ln1", bufs=1))
            ln1_t = ln1.tile([128, KT, T], f8)
            layernorm(ln1_t,
                      lambda kt, ch: ("dma_bulk",
                                      xTb_d.rearrange("(k p) n -> p k n",
                                                      p=128)[
                                          :, :, ch * CH:(ch + 1) * CH]),
                      T, ln1_g, ln1_b)

            qkv_ctx = ExitStack()
            qkvp = qkv_ctx.enter_context(tc.tile_pool(name="qkvp", bufs=1, side="right"))
            qT = qkvp.tile([128, KT, 2 * CH], bf16)
            kT = qkvp.tile([128, KT, T], bf16)
            v_aug = qkvp.tile([128, 16, NH * 65], bf16)
            v4 = v_aug.rearrange("p m (h w) -> p m h w", h=NH)

            with tc.tile_pool(name="wqk", bufs=2) as wqk_pool, \
                 tc.tile_pool(name="qkps", bufs=4, space="PSUM") as qkps:
                # Q (groups 0..3 cover cols 0..1023), K (4..11 -> 1024..3071)
                # fp8 DoubleRow: contraction pairs of k-tiles, PSUM = 1024x
                # true scale (acts x16, weights x64); descale on evacuation.
                for g in range(8):
                    panel = wqk_pool.tile([128, KT, 256], f8, tag="w")
                    nc.sync.dma_start(
                        panel[:],
                        w_attn_d.rearrange("(k p) n -> p k n", p=128)[
                            :, :, g * 256:(g + 1) * 256])
                    for mm in range(2):
                        mt = 2 * g + mm
                        is_q = mt < 8
                        n_chunks = 2 if is_q else 4
                        dst = qT if is_q else kT
                        dt_idx = mt if is_q else mt - 8
                        for nq in range(n_chunks):
                            ps = qkps.tile([128, CH], f32, tag="ps")
                            for kt in range(0, KT, 2):
                                nc.tensor.matmul(
                                    ps[:],
                                    panel[:, kt:kt + 2, mm * 128:(mm + 1) * 128],
                                    ln1_t[:, kt:kt + 2, nq * CH:(nq + 1) * CH],
                                    start=(kt == 0), stop=(kt == KT - 2),
                                    perf_mode=DR)
                            nc.scalar.activation(
                                dst[:, dt_idx, nq * CH:(nq + 1) * CH], ps[:],
                                AF.Identity, bias=b_qk[:, mt, 0:1],
                                scale=2.0 ** -10)
                # V in natural layout, heads interleaved with a ones column
                for g in range(2):
                    panel = wqk_pool.tile([128, KT, CH], f8, tag="wv")
                    nc.sync.dma_start(
                        panel[:],
                        w_attn_d.rearrange("(k p) n -> p k n", p=128)[
                            :, :, 2 * E + g * CH:2 * E + (g + 1) * CH])
                    bv_bc = qkps.tile([128, CH], f32, tag="bv", bufs=2)
                    nc.tensor.matmul(bv_bc[:], ones_row_f[:],
                                     b_v[:, g * CH:(g + 1) * CH],
                                     start=True, stop=True)
                    bv_sb = wqk_pool.tile([128, CH], f32, tag="bvs")
                    nc.vector.tensor_copy(bv_sb[:], bv_bc[:])
                    bv_sb3 = bv_sb.rearrange("p (h w) -> p h w", h=8)
                    for mv in range(16):
                        ps = qkps.tile([128, CH], f32, tag="ps")
                        for kt in range(0, KT, 2):
                            nc.tensor.matmul(
                                ps[:],
                                ln1_t[:, kt:kt + 2, mv * 128:(mv + 1) * 128],
                                panel[:, kt:kt + 2, :],
                                start=(kt == 0), stop=(kt == KT - 2),
                                perf_mode=DR)
                        ps3 = ps.rearrange("p (h w) -> p h w", h=8)
                        nc.vector.scalar_tensor_tensor(
                            v4[:, mv, g * 8:(g + 1) * 8, 0:64], ps3[:],
                            2.0 ** -10, bv_sb3[:], ALU.mult, ALU.add)
                for mv in range(16):
                    nc.vector.memset(v4[:, mv, :, 64:65], 1.0)
            ln1_ctx.close()

            # ---------------- Phase 3: attention ----------------
            # Head-PAIR processing: heads (2j, 2j+1) live on partition rows
            # 0:64 / 64:128 of feature group j, so their score matmuls use
            # disjoint PE row-groups (tile_position auto-derived) and run
            # concurrently. kv tiles are processed in groups of 2 of the same
            # mask kind so one exp ACTIVATE covers [128, 2*CH] (2 PSUM banks).
            # Diag masks are 0/1 multiplies AFTER exp (cheaper: bf16 2x DVE).
            attn_ctx = ExitStack()
            attnp = attn_ctx.enter_context(tc.tile_pool(name="attnp", bufs=1))
            attnT = attnp.tile([128, KT, 2 * CH], bf16)

            # groups: (kv_t0, kv_t1, kind); kind: ("diag", pair) | ("gate", which, idx)
            groups_a = [(0, 1, ("diag", 0)), (2, 3, ("diag", 1)),
                        (8, 9, ("gate", "A", 0)), (10, 11, ("gate", "A", 0))]
            groups_b = [(4, 5, ("diag", 0)), (6, 7, ("diag", 1)),
                        (0, 1, ("gate", "B", 0)), (2, 3, ("gate", "B", 0)),
                        (8, 9, ("gate", "B", 1)), (10, 11, ("gate", "B", 1)),
                        (12, 13, ("gate", "B", 2)), (14, 15, ("gate", "B", 2))]

            with tc.tile_pool(name="atps", bufs=1, space="PSUM") as atps, \
                 tc.tile_pool(name="atsb", bufs=1) as atsb:

                def do_scores(slot, j, t0, t1):
                    qc = slice(slot * CH, (slot + 1) * CH)
                    ss = [atps.tile([128, 2, CH], f32, tag="s",
                                    bufs=3, name=f"s{hh}")
                          for hh in range(2)]
                    for ti, t in enumerate((t0, t1)):
                        for hh in range(2):
                            ro = hh * 64
                            nc.tensor.matmul(
                                ss[hh][:, ti, :],
                                kT[ro:ro + 64, j, t * 128:(t + 1) * 128],
                                qT[ro:ro + 64, j, qc],
                                start=True, stop=True)
                    return ss

                def do_exp_av(slot, last, j, avs, drcs, gi, t0, t1, kind, ss):
                    for hh in range(2):
                        es = atsb.tile([128, 2, CH], bf16, tag="es",
                                       bufs=4)
                        if kind[0] == "diag":
                            er = atsb.tile([128, 2, CH], bf16,
                                           tag="er", bufs=2)
                            nc.scalar.activation(er[:], ss[hh][:], AF.Exp)
                            pr = kind[1]
                            nc.vector.tensor_mul(
                                es[:], er[:],
                                dmask[:, 2 * pr:2 * pr + 2, :])
                        else:
                            sc = sA_s if kind[1] == "A" else sB_s
                            bi = sA_b if kind[1] == "A" else sB_b
                            idx = kind[2]
                            nc.scalar.activation(
                                es[:], ss[hh][:], AF.Exp,
                                bias=bi[:, idx:idx + 1],
                                scale=sc[:, idx:idx + 1])
                        h = 2 * j + hh
                        for ti, t in enumerate((t0, t1)):
                            nc.tensor.matmul(
                                avs[hh][:],
                                v_aug[:, t, h * 65:(h + 1) * 65],
                                es[:, ti, :],
                                start=(gi == 0 and ti == 0),
                                stop=(gi == last and ti == 1))
                        if gi == last:
                            # start the denominator chain now so the norm's
                            # broadcast matmul isn't blocked on DVE later
                            den = atsb.tile([1, CH], f32, tag="den", bufs=4)
                            nc.vector.tensor_copy(den[:], avs[hh][64:65, :])
                            drc = atsb.tile([1, CH], f32, tag="drc", bufs=4)
                            nc.vector.reciprocal_approx_fast(
                                out=drc[:], in_=den[:])
                            drcs.append(drc)

                def do_norm(slot, j, avs, drcs):
                    # PE broadcast of 1/den (f32, steals an s-tag slot), scale
                    qc = slice(slot * CH, (slot + 1) * CH)
                    bct = atps.tile([128, 2, CH], f32, tag="s",
                                    bufs=3, name="bc")
                    for hh in range(2):
                        ro = hh * 64
                        nc.tensor.matmul(bct[0:64, hh, :],
                                         ones_row_f[0:1, 0:64],
                                         drcs[hh][:], start=True, stop=True)
                        bc_sb = atsb.tile([64, CH], f32, tag="bcs",
                                          bufs=2)
                        nc.vector.tensor_copy(bc_sb[:], bct[0:64, hh, :])
                        nc.vector.tensor_mul(
                            attnT[ro:ro + 64, j, qc],
                            avs[hh][0:64, :], bc_sb[:])

                # software-pipelined stream over both slots: scores run
                # 1-2 groups ahead of exp/AV; norms deferred one item more
                work = []
                norms = []
                avs_j = {}
                drcs_j = {}
                stream = [(slot, len(groups) - 1, j, gi, grp)
                          for slot, groups in ((0, groups_a), (1, groups_b))
                          for j in range(8)
                          for gi, grp in enumerate(groups)]
                for slot, last, j, gi, (t0, t1, kind) in stream:
                    sj = (slot, j)
                    if gi == 0:
                        avs_j[sj] = [atps.tile([65, CH], f32, tag="av",
                                               bufs=2, name=f"av{hh}")
                                     for hh in range(2)]
                        drcs_j[sj] = []
                    ss = do_scores(slot, j, t0, t1)
                    if norms:
                        do_norm(*norms.pop(0))
                    work.append((slot, last, j, gi, t0, t1, kind, ss))
                    if len(work) >= 2:
                        sl, la, jj, gg, tt0, tt1, kk, sss = work.pop(0)
                        sjj = (sl, jj)
                        do_exp_av(sl, la, jj, avs_j[sjj], drcs_j[sjj],
                                  gg, tt0, tt1, kk, sss)
                        if gg == la:
                            norms.append((sl, jj, avs_j.pop(sjj),
                                          drcs_j.pop(sjj)))
                for sl, la, jj, gg, tt0, tt1, kk, sss in work:
                    sjj = (sl, jj)
                    do_exp_av(sl, la, jj, avs_j[sjj], drcs_j[sjj],
                              gg, tt0, tt1, kk, sss)
                    if gg == la:
                        norms.append((sl, jj, avs_j.pop(sjj),
                                      drcs_j.pop(sjj)))
                for nrm in norms:
                    do_norm(*nrm)

            qkv_ctx.close()

            # ---------------- Phase 4: attn proj + residual ----------------
            x2p = stack.enter_context(tc.tile_pool(name="x2p", bufs=1, side="right"))
            x2T = x2p.tile([128, KT, 2 * CH], f32)

            # ---------------- Phase 5-7: LN2 + MLP, chunk-pipelined --------
            # Emission order keeps the PE queue dense: LN2(A) stats ride
            # behind attnproj(B); FC(A) runs while LN2(B)'s vector/scalar
            # work proceeds; proj(A) (1-bank m-major form) interleaves with
            # FC(B); proj(B) uses the full-bank kt-major form at the end.
            gp = stack.enter_context(tc.tile_pool(name="gp", bufs=1, side="right"))
            gT = gp.tile([128, 32, 2 * CH], bf16)
            h2p = stack.enter_context(tc.tile_pool(name="h2p", bufs=1,
                                                   side="right"))
            h2T = h2p.tile([128, KT, 2 * CH], bf16)
            ln2_src = lambda kt, ch: ("sbuf_f32",
                                      x2T[:, kt, ch * CH:(ch + 1) * CH])

            ap_ctx = ExitStack()
            app = ap_ctx.enter_context(tc.tile_pool(name="app", bufs=1))
            apsb = ap_ctx.enter_context(tc.tile_pool(name="apsb", bufs=3))
            apps = ap_ctx.enter_context(
                tc.tile_pool(name="apps", bufs=3, space="PSUM"))
            w_ap = app.tile([128, KT, E], bf16)
            nc.sync.dma_start(w_ap[:], w_ap_d.rearrange("(k p) n -> p k n", p=128))

            def attnproj(nq):
                for m in range(KT):
                    ps = apps.tile([128, CH], f32, tag="ps")
                    for kt in range(KT):
                        nc.tensor.matmul(
                            ps[:], w_ap[:, kt, m * 128:(m + 1) * 128],
                            attnT[:, kt, nq * CH:(nq + 1) * CH],
                            start=(kt == 0), stop=(kt == KT - 1))
                    xq = apsb.tile([128, CH], f32, tag="xq")
                    nc.sync.dma_start(
                        xq[:], xT_d[m * 128:(m + 1) * 128,
                                    nq * CH:(nq + 1) * CH])
                    nc.vector.scalar_tensor_tensor(
                        x2T[:, m, nq * CH:(nq + 1) * CH], ps[:],
                        b_ap[:, m, 0:1], xq[:], ALU.add, ALU.add)

            attnproj(0)
            layernorm(h2T, ln2_src, 2 * CH, ln2_g, ln2_b, chunks=[0])
            attnproj(1)
            ap_ctx.close()
            attn_ctx.close()

            fc_ctx = ExitStack()
            wfcp = fc_ctx.enter_context(tc.tile_pool(name="wfcp", bufs=2))
            fcps = fc_ctx.enter_context(
                tc.tile_pool(name="fcps", bufs=4, space="PSUM"))

            def fc_block(mg, nq):
                panel = wfcp.tile([128, KT, CH], bf16, tag="w")
                nc.sync.dma_start(
                    panel[:],
                    w_fc_d.rearrange("(k p) n -> p k n", p=128)[
                        :, :, mg * CH:(mg + 1) * CH])
                for mm in range(4):
                    mt = mg * 4 + mm
                    ps = fcps.tile([128, CH], f32, tag="ps")
                    for kt in range(KT):
                        nc.tensor.matmul(
                            ps[:], panel[:, kt, mm * 128:(mm + 1) * 128],
                            h2T[:, kt, nq * CH:(nq + 1) * CH],
                            start=(kt == 0), stop=(kt == KT - 1))
                    nc.scalar.activation(
                        gT[:, mt, nq * CH:(nq + 1) * CH], ps[:],
                        AF.Gelu, bias=b_fc[:, mt, 0:1])

            for mg in range(8):
                fc_block(mg, 0)
            layernorm(h2T, ln2_src, 2 * CH, ln2_g, ln2_b, chunks=[1])

            prA_ctx = ExitStack()
            wppA = prA_ctx.enter_context(tc.tile_pool(name="wppA", bufs=2))
            prsbA = prA_ctx.enter_context(tc.tile_pool(name="prsbA", bufs=2))
            prpsA = prA_ctx.enter_context(
                tc.tile_pool(name="prpsA", bufs=2, space="PSUM"))

            def proj_m(m, nq):
                panel = wppA.tile([128, 32, 128], bf16, tag="w")
                nc.sync.dma_start(
                    panel[:],
                    w_pr_d[:, m * 128:(m + 1) * 128].rearrange(
                        "(k p) n -> p k n", p=128))
                acc = prpsA.tile([128, CH], f32, tag="ps")
                for kt in range(32):
                    nc.tensor.matmul(
                        acc[:], panel[:, kt, :],
                        gT[:, kt, nq * CH:(nq + 1) * CH],
                        start=(kt == 0), stop=(kt == 31))
                ot = prsbA.tile([128, CH], f32, tag="ot")
                nc.vector.scalar_tensor_tensor(
                    ot[:], acc[:], b_pr[:, m, 0:1],
                    x2T[:, m, nq * CH:(nq + 1) * CH], ALU.add, ALU.add)
                nc.sync.dma_start(
                    out_d[m * 128:(m + 1) * 128, nq * CH:(nq + 1) * CH],
                    ot[:])

            for mg in range(8):
                fc_block(mg, 1)
                proj_m(mg, 0)
            # proj chunk B: same m-major form (evac + out DMA of each
            # m-tile overlap the next m-tile's matmuls)
            for m in range(KT):
                proj_m(m, 1)
            prA_ctx.close()
            fc_ctx.close()

    nc.compile()
    return nc


def _host_prep(inputs):
    """Build the 8 per-core input maps.

    fp8 scaling scheme: weights x64, LN outputs x16 (folded into the LN
    gain/bias) -> GEMM PSUM at 1024x (or 64x where the activation input is
    at true scale); descaled during evacuation.
    """
    x = np.asarray(inputs["x"], np.float32)
    w_attn = np.asarray(inputs["w_attn"], np.float32).copy()
    w_attn[:, :E] *= 0.125  # fold 1/sqrt(head_dim) into Q
    b_attn = np.asarray(inputs["b_attn"], np.float32).copy()
    b_attn[:E] *= 0.125
    f8 = lambda w: np.ascontiguousarray(
        (np.asarray(w, np.float32) * 64.0).astype(F8))
    w_attn_f8 = f8(w_attn)
    b_qk = np.ascontiguousarray(b_attn[:2 * E].reshape(2 * E, 1))
    b_v = np.ascontiguousarray(b_attn[2 * E:].reshape(1, E))
    w_ap_bf = np.ascontiguousarray(
        np.asarray(inputs["w_attnproj"], np.float32).astype(BF))
    w_fc_bf = np.ascontiguousarray(
        np.asarray(inputs["w_fc"], np.float32).astype(BF))
    w_pr_bf = np.ascontiguousarray(
        np.asarray(inputs["w_proj"], np.float32).astype(BF))
    col = lambda v: np.ascontiguousarray(np.asarray(v, np.float32).reshape(-1, 1))
    b_ap = col(inputs["b_attnproj"])
    b_fc = col(inputs["b_fc"])
    b_pr = col(inputs["b_proj"])
    ln1_g = col(inputs["ln1_g"]) * 16.0
    ln1_b = col(inputs["ln1_b"]) * 16.0
    ln2_g = col(inputs["ln2_g"])
    ln2_b = col(inputs["ln2_b"])

    # static diagonal masks (post-exp multiply): 1 if j >= r*128+p else 0
    j = np.arange(CH)[None, :]
    p = np.arange(128)[:, None]
    dmask = np.stack([np.where(j >= r * 128 + p, 1.0, 0.0) for r in range(4)])
    dmask = np.ascontiguousarray(dmask.astype(BF))

    ON = (1.0, 0.0)
    OFF = (0.0, NEG)
    in_maps = []
    perms = []
    for core in range(8):
        b = core // 2
        z = core % 2
        blocks = [0, 3, 1, 2] if z == 0 else [1, 2, 0, 3]
        perms.append(blocks)
        cols = np.concatenate([np.arange(c * CH, (c + 1) * CH) for c in blocks])
        xT = np.ascontiguousarray(x[b].T[:, cols])
        # slot A: driven block = O1 (perm pos 2); allowed iff block(O1) < block(A)
        sa = ON if blocks[2] < blocks[0] else OFF
        # slot B: driven = A, O1, O2 (perm pos 0, 2, 3) vs chunk B
        sbs = [ON if blocks[i] < blocks[1] else OFF for i in (0, 2, 3)]
        f = np.float32
        in_maps.append({
            "xT": xT, "xTb": np.ascontiguousarray(xT.astype(BF)),
            "w_attn": w_attn_f8, "b_qk": b_qk, "b_v": b_v,
            "w_ap": w_ap_bf, "b_ap": b_ap,
            "ln1_g": ln1_g, "ln1_b": ln1_b, "ln2_g": ln2_g, "ln2_b": ln2_b,
            "w_fc": w_fc_bf, "b_fc": b_fc, "w_proj": w_pr_bf, "b_proj": b_pr,
            "dmask": dmask,
            "sA_scale": np.full((128, 1), sa[0], f),
            "sA_bias": np.full((128, 1), sa[1], f),
            "sB_scale": np.ascontiguousarray(
                np.tile(np.array([[s for s, _ in sbs]], f), (128, 1))),
            "sB_bias": np.ascontiguousarray(
                np.tile(np.array([[bb for _, bb in sbs]], f), (128, 1))),
        })
    return in_maps, perms


def _run(inputs, trace=False):
    from concourse.bass_utils import run_bass_kernel_spmd

    if "nc" not in _CACHE:
        _CACHE["nc"] = _build_program()
    nc = _CACHE["nc"]
    in_maps, perms = _host_prep(inputs)
    res = run_bass_kernel_spmd(nc, in_maps, list(range(8)), trace=trace)
    x = np.asarray(inputs["x"], np.float32)
    out = np.empty_like(x)
    for core in range(8):
        b = core // 2
        blocks = perms[core]
        oT = res.results[core]["outT"]
        cA, cB = blocks[0], blocks[1]
        out[b, cA * CH:(cA + 1) * CH, :] = oT[:, 0:CH].T
        out[b, cB * CH:(cB + 1) * CH, :] = oT[:, CH:2 * CH].T
    return out, res


def kernel(**inputs) -> np.ndarray:
    out, _ = _run(inputs, trace=False)
    return out



# revision 82
# speedup vs baseline: 1.0383x; 1.0383x over previous
"""GPT block (LN -> causal MHA -> LN -> MLP) on 8 TRN2 NeuronCores.

Sharding: each core owns one (batch, query-chunk-pair). B=4 batches x 2
chunk-pairs = 8 cores. Chunk pairs are zig-zag ({0,3} / {1,2}) over four
512-row chunks of T=2048 so attention work balances. Each core recomputes
K/V for the full sequence locally (no collectives), runs flash-style
attention for its 1024 query rows, then the MLP for the same rows.

All activations live feature-on-partition ("transposed"); the host
pre-transposes x and assembles the output, so no on-chip transposes are
needed. Per-core causality is handled with a block permutation of the
sequence: each core sees [own-chunk-A | own-chunk-B | other | other], so
the program is identical across cores; the diagonal-block masks are
static and full-block allow/deny is driven by per-core exp scale/bias
inputs (exp(0*s - 1e9) = 0 kills forbidden blocks).
"""

import numpy as np
import ml_dtypes

BF = ml_dtypes.bfloat16
F8 = ml_dtypes.float8_e4m3

E = 1024          # embedding
T = 2048          # sequence
B = 4             # batch
NH = 16           # heads
D = 64            # head dim
HID = 4096        # mlp hidden
KT = E // 128     # k-tiles over embedding (8)
CH = 512          # chunk rows
NEG = -1.0e9
EPS = 1e-5

_CACHE = {}


def _build_program():
    import concourse.bass as bass
    import concourse.tile as tile
    from concourse import bacc, mybir

    f32 = mybir.dt.float32
    bf16 = mybir.dt.bfloat16
    f8 = mybir.dt.float8e4
    AF = mybir.ActivationFunctionType
    ALU = mybir.AluOpType
    DR = mybir.MatmulPerfMode.DoubleRow

    nc = bacc.Bacc()

    xT_d = nc.declare_dram_parameter("xT", [E, T], f32, isOutput=False)
    xTb_d = nc.declare_dram_parameter("xTb", [E, T], bf16, isOutput=False)
    w_attn_d = nc.declare_dram_parameter("w_attn", [E, 3 * E], f8, isOutput=False)
    b_qk_d = nc.declare_dram_parameter("b_qk", [2 * E, 1], f32, isOutput=False)
    b_v_d = nc.declare_dram_parameter("b_v", [1, E], f32, isOutput=False)
    w_ap_d = nc.declare_dram_parameter("w_ap", [E, E], bf16, isOutput=False)
    b_ap_d = nc.declare_dram_parameter("b_ap", [E, 1], f32, isOutput=False)
    ln1_g_d = nc.declare_dram_parameter("ln1_g", [E, 1], f32, isOutput=False)
    ln1_b_d = nc.declare_dram_parameter("ln1_b", [E, 1], f32, isOutput=False)
    ln2_g_d = nc.declare_dram_parameter("ln2_g", [E, 1], f32, isOutput=False)
    ln2_b_d = nc.declare_dram_parameter("ln2_b", [E, 1], f32, isOutput=False)
    w_fc_d = nc.declare_dram_parameter("w_fc", [E, HID], bf16, isOutput=False)
    b_fc_d = nc.declare_dram_parameter("b_fc", [HID, 1], f32, isOutput=False)
    w_pr_d = nc.declare_dram_parameter("w_proj", [HID, E], bf16, isOutput=False)
    b_pr_d = nc.declare_dram_parameter("b_proj", [E, 1], f32, isOutput=False)
    dmask_d = nc.declare_dram_parameter("dmask", [4, 128, CH], bf16, isOutput=False)
    sA_s_d = nc.declare_dram_parameter("sA_scale", [128, 1], f32, isOutput=False)
    sA_b_d = nc.declare_dram_parameter("sA_bias", [128, 1], f32, isOutput=False)
    sB_s_d = nc.declare_dram_parameter("sB_scale", [128, 3], f32, isOutput=False)
    sB_b_d = nc.declare_dram_parameter("sB_bias", [128, 3], f32, isOutput=False)
    out_d = nc.declare_dram_parameter("outT", [E, 2 * CH], f32, isOutput=True)

    with tile.TileContext(nc) as tc:
        from contextlib import ExitStack

        stack = ExitStack()
        with stack:
            const = stack.enter_context(tc.tile_pool(name="const", bufs=1))

            ones_col_bf = const.tile([128, 1], bf16)
            nc.vector.memset(ones_col_bf[:], 1.0)
            ones_row_f = const.tile([1, 128], f32)
            nc.vector.memset(ones_row_f[:], 1.0)
            ones_row_bf = const.tile([1, 64], bf16)
            nc.vector.memset(ones_row_bf[:], 1.0)
            ones_ch_bf = const.tile([1, CH], bf16)
            nc.vector.memset(ones_ch_bf[:], 1.0)
            eps_t = const.tile([1, 1], f32)
            nc.vector.memset(eps_t[:], EPS)

            dmask = const.tile([128, 4, CH], bf16)
            nc.sync.dma_start(dmask[:], dmask_d.rearrange("v p n -> p v n"))
            sA_s = const.tile([128, 1], f32)
            nc.sync.dma_start(sA_s[:], sA_s_d[:])
            sA_b = const.tile([128, 1], f32)
            nc.sync.dma_start(sA_b[:], sA_b_d[:])
            sB_s = const.tile([128, 3], f32)
            nc.sync.dma_start(sB_s[:], sB_s_d[:])
            sB_b = const.tile([128, 3], f32)
            nc.sync.dma_start(sB_b[:], sB_b_d[:])

            ln1_g = const.tile([128, KT, 1], f32)
            nc.sync.dma_start(ln1_g[:], ln1_g_d.rearrange("(k p) o -> p k o", p=128))
            ln1_b = const.tile([128, KT, 1], f32)
            nc.sync.dma_start(ln1_b[:], ln1_b_d.rearrange("(k p) o -> p k o", p=128))
            ln2_g = const.tile([128, KT, 1], f32)
            nc.sync.dma_start(ln2_g[:], ln2_g_d.rearrange("(k p) o -> p k o", p=128))
            ln2_b = const.tile([128, KT, 1], f32)
            nc.sync.dma_start(ln2_b[:], ln2_b_d.rearrange("(k p) o -> p k o", p=128))
            b_qk = const.tile([128, 16, 1], f32)
            nc.sync.dma_start(b_qk[:], b_qk_d.rearrange("(k p) o -> p k o", p=128))
            b_v = const.tile([1, E], f32)
            nc.sync.dma_start(b_v[:], b_v_d[:])
            b_ap = const.tile([128, KT, 1], f32)
            nc.sync.dma_start(b_ap[:], b_ap_d.rearrange("(k p) o -> p k o", p=128))
            b_fc = const.tile([128, 32, 1], f32)
            nc.sync.dma_start(b_fc[:], b_fc_d.rearrange("(k p) o -> p k o", p=128))
            b_pr = const.tile([128, KT, 1], f32)
            nc.sync.dma_start(b_pr[:], b_pr_d.rearrange("(k p) o -> p k o", p=128))

            def layernorm(dst, src_fn, ncols, g_t, b_t, chunks=None):
                """dst[:, kt, c] = LN over feature dim of src columns.

                src_fn(kt, ch) -> ("dma_bf", dram bf16 AP) | ("sbuf_f32", AP)
                All elementwise work runs in bf16 (DVE 2x mode).
                """
                if chunks is None:
                    chunks = range(ncols // CH)
                with tc.tile_pool(name="lnp", bufs=2, space="PSUM") as lnps, \
                     tc.tile_pool(name="lns", bufs=3) as lnsb:
                    for ch in chunks:
                        xbfs = []
                        mu_ps = lnps.tile([1, CH], f32, tag="stat")
                        ss_ps = lnps.tile([1, CH], f32, tag="stat")
                        for kt in range(KT):
                            kind, src = src_fn(kt, ch)
                            xbf = lnsb.tile([128, CH], bf16, tag="xbf",
                                            bufs=10)
                            if kind == "dma_bf":
                                nc.sync.dma_start(xbf[:], src)
                            else:
                                nc.scalar.copy(xbf[:], src)
                            xbfs.append(xbf)
                            sq = lnsb.tile([128, CH], bf16, tag="sq")
                            nc.vector.tensor_mul(sq[:], xbf[:], xbf[:])
                            nc.tensor.matmul(mu_ps[:], ones_col_bf[:], xbf[:],
                                             start=(kt == 0), stop=(kt == KT - 1))
                            nc.tensor.matmul(ss_ps[:], ones_col_bf[:], sq[:],
                                             start=(kt == 0), stop=(kt == KT - 1))
                        # rows: musq = (S1/E)^2; var = S2/E - musq;
                        # a = 1/sqrt(var+eps); c = (-S1/E)*a
                        musq = lnsb.tile([1, CH], f32, tag="row", bufs=8)
                        nc.scalar.activation(musq[:], mu_ps[:], AF.Square,
                                             scale=1.0 / E)
                        var = lnsb.tile([1, CH], f32, tag="row", bufs=8)
                        nc.vector.scalar_tensor_tensor(
                            var[:], ss_ps[:], 1.0 / E, musq[:],
                            ALU.mult, ALU.subtract)
                        sd = lnsb.tile([1, CH], f32, tag="row", bufs=8)
                        nc.scalar.activation(sd[:], var[:], AF.Sqrt,
                                             bias=eps_t[:])
                        a_row = lnsb.tile([1, CH], f32, tag="row", bufs=8)
                        nc.vector.reciprocal_approx_fast(out=a_row[:], in_=sd[:])
                        c_row = lnsb.tile([1, CH], f32, tag="row", bufs=8)
                        nc.vector.scalar_tensor_tensor(
                            c_row[:], mu_ps[:], -1.0 / E, a_row[:],
                            ALU.mult, ALU.mult)
                        a_bc = lnps.tile([128, CH], f32, tag="bc")
                        nc.tensor.matmul(a_bc[:], ones_row_f[:], a_row[:],
                                         start=True, stop=True)
                        c_bc = lnps.tile([128, CH], f32, tag="bc")
                        nc.tensor.matmul(c_bc[:], ones_row_f[:], c_row[:],
                                         start=True, stop=True)
                        a_sb = lnsb.tile([128, CH], bf16, tag="asb")
                        nc.vector.tensor_copy(a_sb[:], a_bc[:])
                        c_sb = lnsb.tile([128, CH], bf16, tag="csb")
                        nc.vector.tensor_copy(c_sb[:], c_bc[:])
                        for kt in range(KT):
                            t1 = lnsb.tile([128, CH], bf16, tag="t1")
                            nc.vector.tensor_mul(t1[:], xbfs[kt][:], a_sb[:])
                            t2 = lnsb.tile([128, CH], bf16, tag="t2")
                            nc.vector.tensor_add(t2[:], t1[:], c_sb[:])
                            # gain/bias apply on ScalarE (idle during LN)
                            nc.scalar.activation(
                                dst[:, kt, ch * CH:(ch + 1) * CH], t2[:],
                                AF.Identity, bias=b_t[:, kt, 0:1],
                                scale=g_t[:, kt, 0:1])

            # ---------------- Phase 1+2: LN1 and QKV ----------------
            ln1_ctx = ExitStack()
            ln1 = ln1_ctx.enter_context(tc.tile_pool(name="ln1", bufs=1))
            ln1_t = ln1.tile([128, KT, T], f8)
            layernorm(ln1_t,
                      lambda kt, ch: ("dma_bf",
                                      xTb_d[kt * 128:(kt + 1) * 128,
                                            ch * CH:(ch + 1) * CH]),
                      T, ln1_g, ln1_b)

            qkv_ctx = ExitStack()
            qkvp = qkv_ctx.enter_context(tc.tile_pool(name="qkvp", bufs=1, side="right"))
            qT = qkvp.tile([128, KT, 2 * CH], bf16)
            kT = qkvp.tile([128, KT, T], bf16)
            v_aug = qkvp.tile([128, 16, NH * 65], bf16)
            v4 = v_aug.rearrange("p m (h w) -> p m h w", h=NH)

            with tc.tile_pool(name="wqk", bufs=2) as wqk_pool, \
                 tc.tile_pool(name="qkps", bufs=4, space="PSUM") as qkps:
                # Q (groups 0..3 cover cols 0..1023), K (4..11 -> 1024..3071)
                # fp8 DoubleRow: contraction pairs of k-tiles, PSUM = 1024x
                # true scale (acts x16, weights x64); descale on evacuation.
                for g in range(8):
                    panel = wqk_pool.tile([128, KT, 256], f8, tag="w")
                    nc.sync.dma_start(
                        panel[:],
                        w_attn_d.rearrange("(k p) n -> p k n", p=128)[
                            :, :, g * 256:(g + 1) * 256])
                    for mm in range(2):
                        mt = 2 * g + mm
                        is_q = mt < 8
                        n_chunks = 2 if is_q else 4
                        dst = qT if is_q else kT
                        dt_idx = mt if is_q else mt - 8
                        for nq in range(n_chunks):
                            ps = qkps.tile([128, CH], f32, tag="ps")
                            for kt in range(0, KT, 2):
                                nc.tensor.matmul(
                                    ps[:],
                                    panel[:, kt:kt + 2, mm * 128:(mm + 1) * 128],
                                    ln1_t[:, kt:kt + 2, nq * CH:(nq + 1) * CH],
                                    start=(kt == 0), stop=(kt == KT - 2),
                                    perf_mode=DR)
                            nc.scalar.activation(
                                dst[:, dt_idx, nq * CH:(nq + 1) * CH], ps[:],
                                AF.Identity, bias=b_qk[:, mt, 0:1],
                                scale=2.0 ** -10)
                # V in natural layout, heads interleaved with a ones column
                for g in range(2):
                    panel = wqk_pool.tile([128, KT, CH], f8, tag="wv")
                    nc.sync.dma_start(
                        panel[:],
                        w_attn_d.rearrange("(k p) n -> p k n", p=128)[
                            :, :, 2 * E + g * CH:2 * E + (g + 1) * CH])
                    bv_bc = qkps.tile([128, CH], f32, tag="bv", bufs=2)
                    nc.tensor.matmul(bv_bc[:], ones_row_f[:],
                                     b_v[:, g * CH:(g + 1) * CH],
                                     start=True, stop=True)
                    bv_sb = wqk_pool.tile([128, CH], f32, tag="bvs")
                    nc.vector.tensor_copy(bv_sb[:], bv_bc[:])
                    bv_sb3 = bv_sb.rearrange("p (h w) -> p h w", h=8)
                    for mv in range(16):
                        ps = qkps.tile([128, CH], f32, tag="ps")
                        for kt in range(0, KT, 2):
                            nc.tensor.matmul(
                                ps[:],
                                ln1_t[:, kt:kt + 2, mv * 128:(mv + 1) * 128],
                                panel[:, kt:kt + 2, :],
                                start=(kt == 0), stop=(kt == KT - 2),
                                perf_mode=DR)
                        ps3 = ps.rearrange("p (h w) -> p h w", h=8)
                        nc.vector.scalar_tensor_tensor(
                            v4[:, mv, g * 8:(g + 1) * 8, 0:64], ps3[:],
                            2.0 ** -10, bv_sb3[:], ALU.mult, ALU.add)
                for mv in range(16):
                    nc.vector.memset(v4[:, mv, :, 64:65], 1.0)
            ln1_ctx.close()

            # ---------------- Phase 3: attention ----------------
            # Head-PAIR processing: heads (2j, 2j+1) live on partition rows
            # 0:64 / 64:128 of feature group j, so their score matmuls use
            # disjoint PE row-groups (tile_position auto-derived) and run
            # concurrently. kv tiles are processed in groups of 2 of the same
            # mask kind so one exp ACTIVATE covers [128, 2*CH] (2 PSUM banks).
            # Diag masks are 0/1 multiplies AFTER exp (cheaper: bf16 2x DVE).
            attn_ctx = ExitStack()
            attnp = attn_ctx.enter_context(tc.tile_pool(name="attnp", bufs=1))
            attnT = attnp.tile([128, KT, 2 * CH], bf16)

            # groups: (kv_t0, kv_t1, kind); kind: ("diag", pair) | ("gate", which, idx)
            groups_a = [(0, 1, ("diag", 0)), (2, 3, ("diag", 1)),
                        (8, 9, ("gate", "A", 0)), (10, 11, ("gate", "A", 0))]
            groups_b = [(4, 5, ("diag", 0)), (6, 7, ("diag", 1)),
                        (0, 1, ("gate", "B", 0)), (2, 3, ("gate", "B", 0)),
                        (8, 9, ("gate", "B", 1)), (10, 11, ("gate", "B", 1)),
                        (12, 13, ("gate", "B", 2)), (14, 15, ("gate", "B", 2))]

            with tc.tile_pool(name="atps", bufs=1, space="PSUM") as atps, \
                 tc.tile_pool(name="atsb", bufs=1) as atsb:

                def do_scores(slot, j, t0, t1):
                    qc = slice(slot * CH, (slot + 1) * CH)
                    ss = [atps.tile([128, 2, CH], f32, tag="s",
                                    bufs=3, name=f"s{hh}")
                          for hh in range(2)]
                    for ti, t in enumerate((t0, t1)):
                        for hh in range(2):
                            ro = hh * 64
                            nc.tensor.matmul(
                                ss[hh][:, ti, :],
                                kT[ro:ro + 64, j, t * 128:(t + 1) * 128],
                                qT[ro:ro + 64, j, qc],
                                start=True, stop=True)
                    return ss

                def do_exp_av(slot, last, j, avs, drcs, gi, t0, t1, kind, ss):
                    for hh in range(2):
                        es = atsb.tile([128, 2, CH], bf16, tag="es",
                                       bufs=4)
                        if kind[0] == "diag":
                            er = atsb.tile([128, 2, CH], bf16,
                                           tag="er", bufs=2)
                            nc.scalar.activation(er[:], ss[hh][:], AF.Exp)
                            pr = kind[1]
                            nc.vector.tensor_mul(
                                es[:], er[:],
                                dmask[:, 2 * pr:2 * pr + 2, :])
                        else:
                            sc = sA_s if kind[1] == "A" else sB_s
                            bi = sA_b if kind[1] == "A" else sB_b
                            idx = kind[2]
                            nc.scalar.activation(
                                es[:], ss[hh][:], AF.Exp,
                                bias=bi[:, idx:idx + 1],
                                scale=sc[:, idx:idx + 1])
                        h = 2 * j + hh
                        for ti, t in enumerate((t0, t1)):
                            nc.tensor.matmul(
                                avs[hh][:],
                                v_aug[:, t, h * 65:(h + 1) * 65],
                                es[:, ti, :],
                                start=(gi == 0 and ti == 0),
                                stop=(gi == last and ti == 1))
                        if gi == last:
                            # start the denominator chain now so the norm's
                            # broadcast matmul isn't blocked on DVE later
                            den = atsb.tile([1, CH], f32, tag="den", bufs=4)
                            nc.vector.tensor_copy(den[:], avs[hh][64:65, :])
                            drc = atsb.tile([1, CH], f32, tag="drc", bufs=4)
                            nc.vector.reciprocal_approx_fast(
                                out=drc[:], in_=den[:])
                            drcs.append(drc)

                def do_norm(slot, j, avs, drcs):
                    # PE broadcast of 1/den (f32, steals an s-tag slot), scale
                    qc = slice(slot * CH, (slot + 1) * CH)
                    bct = atps.tile([128, 2, CH], f32, tag="s",
                                    bufs=3, name="bc")
                    for hh in range(2):
                        ro = hh * 64
                        nc.tensor.matmul(bct[0:64, hh, :],
                                         ones_row_f[0:1, 0:64],
                                         drcs[hh][:], start=True, stop=True)
                        bc_sb = atsb.tile([64, CH], f32, tag="bcs",
                                          bufs=2)
                        nc.vector.tensor_copy(bc_sb[:], bct[0:64, hh, :])
                        nc.vector.tensor_mul(
                            attnT[ro:ro + 64, j, qc],
                            avs[hh][0:64, :], bc_sb[:])

                # software-pipelined stream over both slots: scores run
                # 1-2 groups ahead of exp/AV; norms deferred one item more
                work = []
                norms = []
                avs_j = {}
                drcs_j = {}
                stream = [(slot, len(groups) - 1, j, gi, grp)
                          for slot, groups in ((0, groups_a), (1, groups_b))
                          for j in range(8)
                          for gi, grp in enumerate(groups)]
                for slot, last, j, gi, (t0, t1, kind) in stream:
                    sj = (slot, j)
                    if gi == 0:
                        avs_j[sj] = [atps.tile([65, CH], f32, tag="av",
                                               bufs=2, name=f"av{hh}")
                                     for hh in range(2)]
                        drcs_j[sj] = []
                    ss = do_scores(slot, j, t0, t1)
                    if norms:
                        do_norm(*norms.pop(0))
                    work.append((slot, last, j, gi, t0, t1, kind, ss))
                    if len(work) >= 2:
                        sl, la, jj, gg, tt0, tt1, kk, sss = work.pop(0)
                        sjj = (sl, jj)
                        do_exp_av(sl, la, jj, avs_j[sjj], drcs_j[sjj],
                                  gg, tt0, tt1, kk, sss)
                        if gg == la:
                            norms.append((sl, jj, avs_j.pop(sjj),
                                          drcs_j.pop(sjj)))
                for sl, la, jj, gg, tt0, tt1, kk, sss in work:
                    sjj = (sl, jj)
                    do_exp_av(sl, la, jj, avs_j[sjj], drcs_j[sjj],
                              gg, tt0, tt1, kk, sss)
                    if gg == la:
                        norms.append((sl, jj, avs_j.pop(sjj),
                                      drcs_j.pop(sjj)))
                for nrm in norms:
                    do_norm(*nrm)

            qkv_ctx.close()

            # ---------------- Phase 4: attn proj + residual ----------------
            x2p = stack.enter_context(tc.tile_pool(name="x2p", bufs=1, side="right"))
            x2T = x2p.tile([128, KT, 2 * CH], f32)

            # ---------------- Phase 5-7: LN2 + MLP, chunk-pipelined --------
            # Emission order keeps the PE queue dense: LN2(A) stats ride
            # behind attnproj(B); FC(A) runs while LN2(B)'s vector/scalar
            # work proceeds; proj(A) (1-bank m-major form) interleaves with
            # FC(B); proj(B) uses the full-bank kt-major form at the end.
            gp = stack.enter_context(tc.tile_pool(name="gp", bufs=1, side="right"))
            gT = gp.tile([128, 32, 2 * CH], bf16)
            h2p = stack.enter_context(tc.tile_pool(name="h2p", bufs=1,
                                                   side="right"))
            h2T = h2p.tile([128, KT, 2 * CH], bf16)
            ln2_src = lambda kt, ch: ("sbuf_f32",
                                      x2T[:, kt, ch * CH:(ch + 1) * CH])

            ap_ctx = ExitStack()
            app = ap_ctx.enter_context(tc.tile_pool(name="app", bufs=1))
            apsb = ap_ctx.enter_context(tc.tile_pool(name="apsb", bufs=3))
            apps = ap_ctx.enter_context(
                tc.tile_pool(name="apps", bufs=3, space="PSUM"))
            w_ap = app.tile([128, KT, E], bf16)
            nc.sync.dma_start(w_ap[:], w_ap_d.rearrange("(k p) n -> p k n", p=128))

            def attnproj(nq):
                for m in range(KT):
                    ps = apps.tile([128, CH], f32, tag="ps")
                    for kt in range(KT):
                        nc.tensor.matmul(
                            ps[:], w_ap[:, kt, m * 128:(m + 1) * 128],
                            attnT[:, kt, nq * CH:(nq + 1) * CH],
                            start=(kt == 0), stop=(kt == KT - 1))
                    xq = apsb.tile([128, CH], f32, tag="xq")
                    nc.sync.dma_start(
                        xq[:], xT_d[m * 128:(m + 1) * 128,
                                    nq * CH:(nq + 1) * CH])
                    nc.vector.scalar_tensor_tensor(
                        x2T[:, m, nq * CH:(nq + 1) * CH], ps[:],
                        b_ap[:, m, 0:1], xq[:], ALU.add, ALU.add)

            attnproj(0)
            layernorm(h2T, ln2_src, 2 * CH, ln2_g, ln2_b, chunks=[0])
            attnproj(1)
            ap_ctx.close()
            attn_ctx.close()

            fc_ctx = ExitStack()
            wfcp = fc_ctx.enter_context(tc.tile_pool(name="wfcp", bufs=2))
            fcps = fc_ctx.enter_context(
                tc.tile_pool(name="fcps", bufs=4, space="PSUM"))

            def fc_block(mg, nq):
                panel = wfcp.tile([128, KT, CH], bf16, tag="w")
                nc.sync.dma_start(
                    panel[:],
                    w_fc_d.rearrange("(k p) n -> p k n", p=128)[
                        :, :, mg * CH:(mg + 1) * CH])
                for mm in range(4):
                    mt = mg * 4 + mm
                    ps = fcps.tile([128, CH], f32, tag="ps")
                    for kt in range(KT):
                        nc.tensor.matmul(
                            ps[:], panel[:, kt, mm * 128:(mm + 1) * 128],
                            h2T[:, kt, nq * CH:(nq + 1) * CH],
                            start=(kt == 0), stop=(kt == KT - 1))
                    nc.scalar.activation(
                        gT[:, mt, nq * CH:(nq + 1) * CH], ps[:],
                        AF.Gelu, bias=b_fc[:, mt, 0:1])

            for mg in range(8):
                fc_block(mg, 0)
            layernorm(h2T, ln2_src, 2 * CH, ln2_g, ln2_b, chunks=[1])

            prA_ctx = ExitStack()
            wppA = prA_ctx.enter_context(tc.tile_pool(name="wppA", bufs=2))
            prsbA = prA_ctx.enter_context(tc.tile_pool(name="prsbA", bufs=2))
            prpsA = prA_ctx.enter_context(
                tc.tile_pool(name="prpsA", bufs=2, space="PSUM"))

            def proj_m(m, nq):
                panel = wppA.tile([128, 32, 128], bf16, tag="w")
                nc.sync.dma_start(
                    panel[:],
                    w_pr_d[:, m * 128:(m + 1) * 128].rearrange(
                        "(k p) n -> p k n", p=128))
                acc = prpsA.tile([128, CH], f32, tag="ps")
                for kt in range(32):
                    nc.tensor.matmul(
                        acc[:], panel[:, kt, :],
                        gT[:, kt, nq * CH:(nq + 1) * CH],
                        start=(kt == 0), stop=(kt == 31))
                ot = prsbA.tile([128, CH], f32, tag="ot")
                nc.vector.scalar_tensor_tensor(
                    ot[:], acc[:], b_pr[:, m, 0:1],
                    x2T[:, m, nq * CH:(nq + 1) * CH], ALU.add, ALU.add)
                nc.sync.dma_start(
                    out_d[m * 128:(m + 1) * 128, nq * CH:(nq + 1) * CH],
                    ot[:])

            for mg in range(8):
                fc_block(mg, 1)
                proj_m(mg, 0)
            # proj chunk B: same m-major form (evac + out DMA of each
            # m-tile overlap the next m-tile's matmuls)
            for m in range(KT):
                proj_m(m, 1)
            prA_ctx.close()
            fc_ctx.close()

    nc.compile()
    return nc


def _host_prep(inputs):
    """Build the 8 per-core input maps.

    fp8 scaling scheme: weights x64, LN outputs x16 (folded into the LN
    gain/bias) -> GEMM PSUM at 1024x (or 64x where the activation input is
    at true scale); descaled during evacuation.
    """
    x = np.asarray(inputs["x"], np.float32)
    w_attn = np.asarray(inputs["w_attn"], np.float32).copy()
    w_attn[:, :E] *= 0.125  # fold 1/sqrt(head_dim) into Q
    b_attn = np.asarray(inputs["b_attn"], np.float32).copy()
    b_attn[:E] *= 0.125
    f8 = lambda w: np.ascontiguousarray(
        (np.asarray(w, np.float32) * 64.0).astype(F8))
    w_attn_f8 = f8(w_attn)
    b_qk = np.ascontiguousarray(b_attn[:2 * E].reshape(2 * E, 1))
    b_v = np.ascontiguousarray(b_attn[2 * E:].reshape(1, E))
    w_ap_bf = np.ascontiguousarray(
        np.asarray(inputs["w_attnproj"], np.float32).astype(BF))
    w_fc_bf = np.ascontiguousarray(
        np.asarray(inputs["w_fc"], np.float32).astype(BF))
    w_pr_bf = np.ascontiguousarray(
        np.asarray(inputs["w_proj"], np.float32).astype(BF))
    col = lambda v: np.ascontiguousarray(np.asarray(v, np.float32).reshape(-1, 1))
    b_ap = col(inputs["b_attnproj"])
    b_fc = col(inputs["b_fc"])
    b_pr = col(inputs["b_proj"])
    ln1_g = col(inputs["ln1_g"]) * 16.0
    ln1_b = col(inputs["ln1_b"]) * 16.0
    ln2_g = col(inputs["ln2_g"])
    ln2_b = col(inputs["ln2_b"])

    # static diagonal masks (post-exp multiply): 1 if j >= r*128+p else 0
    j = np.arange(CH)[None, :]
    p = np.arange(128)[:, None]
    dmask = np.stack([np.where(j >= r * 128 + p, 1.0, 0.0) for r in range(4)])
    dmask = np.ascontiguousarray(dmask.astype(BF))

    ON = (1.0, 0.0)
    OFF = (0.0, NEG)
    in_maps = []
    perms = []
    for core in range(8):
        b = core // 2
        z = core % 2
        blocks = [0, 3, 1, 2] if z == 0 else [1, 2, 0, 3]
        perms.append(blocks)
        cols = np.concatenate([np.arange(c * CH, (c + 1) * CH) for c in blocks])
        xT = np.ascontiguousarray(x[b].T[:, cols])
        # slot A: driven block = O1 (perm pos 2); allowed iff block(O1) < block(A)
        sa = ON if blocks[2] < blocks[0] else OFF
        # slot B: driven = A, O1, O2 (perm pos 0, 2, 3) vs chunk B
        sbs = [ON if blocks[i] < blocks[1] else OFF for i in (0, 2, 3)]
        f = np.float32
        in_maps.append({
            "xT": xT, "xTb": np.ascontiguousarray(xT.astype(BF)),
            "w_attn": w_attn_f8, "b_qk": b_qk, "b_v": b_v,
            "w_ap": w_ap_bf, "b_ap": b_ap,
            "ln1_g": ln1_g, "ln1_b": ln1_b, "ln2_g": ln2_g, "ln2_b": ln2_b,
            "w_fc": w_fc_bf, "b_fc": b_fc, "w_proj": w_pr_bf, "b_proj": b_pr,
            "dmask": dmask,
            "sA_scale": np.full((128, 1), sa[0], f),
            "sA_bias": np.full((128, 1), sa[1], f),
            "sB_scale": np.ascontiguousarray(
                np.tile(np.array([[s for s, _ in sbs]], f), (128, 1))),
            "sB_bias": np.ascontiguousarray(
                np.tile(np.array([[bb for _, bb in sbs]], f), (128, 1))),
        })
    return in_maps, perms


def _run(inputs, trace=False):
    from concourse.bass_utils import run_bass_kernel_spmd

    if "nc" not in _CACHE:
        _CACHE["nc"] = _build_program()
    nc = _CACHE["nc"]
    in_maps, perms = _host_prep(inputs)
    res = run_bass_kernel_spmd(nc, in_maps, list(range(8)), trace=trace)
    x = np.asarray(inputs["x"], np.float32)
    out = np.empty_like(x)
    for core in range(8):
        b = core // 2
        blocks = perms[core]
        oT = res.results[core]["outT"]
        cA, cB = blocks[0], blocks[1]
        out[b, cA * CH:(cA + 1) * CH, :] = oT[:, 0:CH].T
        out[b, cB * CH:(cB + 1) * CH, :] = oT[:, CH:2 * CH].T
    return out, res


def kernel(**inputs) -> np.ndarray:
    out, _ = _run(inputs, trace=False)
    return out

